# revision 12
# baseline (speedup 1.0000x reference)
"""Causal GQA self-attention (B=1, T=2048, C=1024, 16 q-heads, 4 kv-groups, d=64)
on 8 Trainium2 NeuronCores.

Sharding: tensor-parallel over heads. Core c owns q-heads (2c, 2c+1) and kv-group
c//2. Each core computes x @ w_attn for its slice (transposed layout), RoPE,
causal flash-style attention for its 2 heads, and its partial y @ w_proj
(contracting only its 128 head-dims). Host sums the 8 partial outputs.

Layout strategy (per core):
  - xT [C, T] in SBUF (strided DMA); wqkv slice [C, 256] natural.
  - qkvT = wqkv.T @ x computed transposed: qT2 [128, T] (2 heads), kvT [128, T]
    (k rows 0:64, v rows 64:128).
  - RoPE applied in [d, T] layout; the pair-rotation is a PE matmul with a
    constant +-1 permutation matrix; cos/sin are inline (baked) tables.
  - Scores computed TRANSPOSED: sT[k, q] = kT.T-free matmul, so the softmax
    denominator comes from appending a ones-column to v (one extra PE row) and
    no max-subtraction is needed (|scores| <= ~7, exp is safe in fp32).
  - att @ v computed as yT[d, q] via lhsT=v_aug, rhs=pT -- no transposes of p.
  - Normalization folded in before proj via a PE outer-product broadcast.
  - proj: out[t, :] += yT_h.T @ wproj_h per head, accumulated in PSUM.
"""

import numpy as np

import concourse.bass as bass
import concourse.mybir as mybir
from concourse import bacc
import concourse.tile as tile
from concourse.bass_utils import run_bass_kernel_spmd

T = 2048
C = 1024
D = 64
QW = 512                      # queries processed per attention window
NCH = T // 512                # 512-wide column chunks of T
F32 = mybir.dt.float32
F32R = mybir.dt.float32r
EXP = mybir.ActivationFunctionType.Exp
MUL = mybir.AluOpType.mult
ADD = mybir.AluOpType.add

# True: load xT via strided DMA (512B contiguous runs in DRAM).
# False: load x naturally and transpose 128x128 blocks on the PE.
STRIDED_XT = True

_CACHE: dict = {}


def _rope_tables():
    # Replicate reference.apply_rope's f32 pipeline exactly.
    inv = (1.0 / (np.float32(10000.0) ** (np.arange(0, D, 2, dtype=np.float32) / np.float32(D)))).astype(np.float32)
    freqs = (np.arange(T, dtype=np.float32)[:, None] * inv[None, :]).astype(np.float32)  # (T, 32)
    freqs = np.repeat(freqs, 2, axis=1)                                                  # (T, 64)
    cos = np.cos(freqs).astype(np.float32).T.copy()                                      # (64, T)
    sin = np.sin(freqs).astype(np.float32).T.copy()
    cos2 = np.ascontiguousarray(np.concatenate([cos, cos], axis=0))                      # (128, T)
    sin2 = np.ascontiguousarray(np.concatenate([sin, sin], axis=0))
    return cos2, sin2


def _const_mats():
    # perm (as lhsT): rot[2i] = -x[2i+1], rot[2i+1] = +x[2i]
    perm = np.zeros((128, 128), np.float32)
    for i in range(64):
        perm[2 * i + 1, 2 * i] = -1.0
        perm[2 * i, 2 * i + 1] = 1.0
    ident = np.eye(128, dtype=np.float32)
    shift = np.zeros((128, 128), np.float32)   # [64+i, i] = 1 down-shift; [i, 64+i] = 1 up-shift
    for i in range(64):
        shift[64 + i, i] = 1.0
        shift[i, 64 + i] = 1.0
    kq = np.arange(128)
    binmask = (kq[:, None] <= kq[None, :]).astype(np.float32)  # [k, q]: 1 where k <= q
    return perm, ident, shift, binmask


def _build_bass() -> bass.Bass:
    nc = bacc.Bacc(None, target_bir_lowering=False)
    xt_d = nc.dram_tensor("xt", [C, T], F32R, kind="ExternalInput")
    wqkv = nc.dram_tensor("wqkv", [C, 4 * D], F32R, kind="ExternalInput")
    wproj = nc.dram_tensor("wproj", [2 * D, C], F32R, kind="ExternalInput")
    out = nc.dram_tensor("out", [T, C], F32, kind="ExternalOutput")

    cos2_np, sin2_np = _rope_tables()
    perm_np, ident_np, shift_np, binmask_np = _const_mats()
    cos_d = nc.inline_tensor(cos2_np, name="cos2")
    sin_d = nc.inline_tensor(sin2_np, name="sin2")
    perm_d = nc.inline_tensor(perm_np, name="permm")
    ident_d = nc.inline_tensor(ident_np, name="identm")
    shift_d = nc.inline_tensor(shift_np, name="shiftm")
    mask_d = nc.inline_tensor(binmask_np, name="binmaskm")

    with tile.TileContext(nc) as tc:
        with (
            nc.allow_low_precision(reason="fp32r rounding of matmul operands"),
            tc.tile_pool(name="const", bufs=1) as const,
            tc.tile_pool(name="big", bufs=1) as big,
        ):
            # weights first (everything depends on them)
            w_r = const.tile([128, 8, 4 * D], F32R)
            nc.sync.dma_start(out=w_r, in_=wqkv.rearrange("(c p) n -> p c n", p=128))
            wp0r = const.tile([64, C], F32R)
            nc.sync.dma_start(out=wp0r, in_=wproj[0:64, :])
            wp1r = const.tile([64, C], F32R)
            nc.sync.dma_start(out=wp1r, in_=wproj[64:128, :])

            cos_sb = const.tile([128, T], F32)
            nc.sync.dma_start(out=cos_sb, in_=cos_d[:, :])
            sin_sb = const.tile([128, T], F32)
            nc.sync.dma_start(out=sin_sb, in_=sin_d[:, :])
            perm_sb = const.tile([128, 128], F32)
            nc.sync.dma_start(out=perm_sb, in_=perm_d[:, :])
            id_sb = const.tile([128, 128], F32)
            nc.sync.dma_start(out=id_sb, in_=ident_d[:, :])
            sh_sb = const.tile([128, 128], F32)
            nc.sync.dma_start(out=sh_sb, in_=shift_d[:, :])
            mk_sb = const.tile([128, 128], F32)
            nc.sync.dma_start(out=mk_sb, in_=mask_d[:, :])
            ones_f = const.tile([128, 64], F32)
            nc.vector.memset(ones_f, 1.0)
            ones_sb = const.tile([65, 64], F32R)
            nc.vector.tensor_copy(out=ones_sb, in_=ones_f[0:65, :])

            # persistent per-core activations
            qraw_sb = big.tile([128, T], F32)    # [q0|q1]^T raw
            kvraw_sb = big.tile([128, T], F32)   # k^T rows 0:64, v^T rows 64:128
            qrope_sb = big.tile([128, T], F32R)   # roped q, h0 rows 0:64, h1 rows 64:128
            q1_sb = big.tile([64, T], F32R)       # roped q of h1 shifted to partitions 0:64
            krope_sb = big.tile([64, T], F32R)
            tmp_sb = big.tile([128, T], F32)
            tmpk_sb = big.tile([64, T], F32)
            vaug_sb = big.tile([128, 16, D + 1], F32R)  # v tiles + ones column

            # ---------------- stage 1: qkvT + rope + v transpose ----------------
            with (
                tc.tile_pool(name="xp", bufs=1) as xp,
                tc.tile_pool(name="ps1", bufs=1, space="PSUM") as ps1,
            ):
                xt = xp.tile([128, 8, T], F32R)  # x^T as 8 c-tiles (host-transposed input)
                for c in range(8):
                    nc.sync.dma_start(out=xt[:, c, :], in_=xt_d[128 * c : 128 * (c + 1), :])

                # qkvT: m=0 -> qraw, m=1 -> kvraw
                for m, dst in ((0, qraw_sb), (1, kvraw_sb)):
                    for nch in range(NCH):
                        ps = ps1.tile([128, 512], F32, tag="qkv", bufs=2)
                        for c in range(8):
                            nc.tensor.matmul(
                                ps,
                                lhsT=w_r[:, c, 128 * m : 128 * (m + 1)],
                                rhs=xt[:, c, 512 * nch : 512 * (nch + 1)].bitcast(F32R),
                                start=(c == 0),
                                stop=(c == 7),
                            )
                        nc.scalar.copy(out=dst[:, 512 * nch : 512 * (nch + 1)], in_=ps)

                for nch in range(NCH):
                    sl = slice(512 * nch, 512 * (nch + 1))
                    # rope q (both heads at once)
                    rps = ps1.tile([128, 512], F32, tag="rot", bufs=2)
                    nc.tensor.matmul(rps, lhsT=perm_sb, rhs=qraw_sb[:, sl], start=True, stop=True)
                    nc.vector.tensor_mul(tmp_sb[:, sl], rps, sin_sb[:, sl])
                    nc.vector.tensor_mul(qrope_sb[:, sl], qraw_sb[:, sl], cos_sb[:, sl])
                    nc.vector.tensor_add(qrope_sb[:, sl], qrope_sb[:, sl], tmp_sb[:, sl])
                    # rope k (rows 0:64 of kvraw)
                    rpsk = ps1.tile([64, 512], F32, tag="rotk", bufs=1)
                    nc.tensor.matmul(rpsk, lhsT=perm_sb[0:64, 0:64], rhs=kvraw_sb[0:64, sl], start=True, stop=True)
                    nc.vector.tensor_mul(tmpk_sb[:, sl], rpsk, sin_sb[0:64, sl])
                    nc.vector.tensor_mul(krope_sb[:, sl], kvraw_sb[0:64, sl], cos_sb[0:64, sl])
                    nc.vector.tensor_add(krope_sb[:, sl], krope_sb[:, sl], tmpk_sb[:, sl])
                    # shift roped h1 q down to partitions 0:64
                    sps = ps1.tile([64, 512], F32, tag="shift", bufs=1)
                    nc.tensor.matmul(sps, lhsT=sh_sb[64:128, 0:64], rhs=qrope_sb[64:128, sl].bitcast(F32), start=True, stop=True)
                    nc.vector.tensor_copy(out=q1_sb[:, sl], in_=sps)

                # v_aug: transpose v tiles, append ones column
                for tt in range(16):
                    vps = ps1.tile([128, 64], F32, tag="vtr", bufs=2)
                    nc.tensor.transpose(
                        vps,
                        in_=kvraw_sb[64:128, 128 * tt : 128 * (tt + 1)],
                        identity=id_sb[64:128, 64:128],
                    )
                    nc.vector.tensor_copy(out=vaug_sb[:, tt, 0:64], in_=vps)
                nc.vector.tensor_copy(out=vaug_sb[:, :, 64], in_=ones_f[:, 0:16])

            # ---------------- stage 2: attention + proj ----------------
            with (
                tc.tile_pool(name="ps2", bufs=1, space="PSUM") as ps2,
                tc.tile_pool(name="patt", bufs=1) as patt,
            ):
                NQT = T // QW
                for i4 in range(NQT):
                    yn = {}
                    for h in range(2):
                        qsrc = qrope_sb if h == 0 else q1_sb
                        ktiles = 4 * i4 + 4
                        yps = ps2.tile([65, QW], F32, tag="yt", bufs=2)
                        for j in range(ktiles):
                            g = j - 4 * i4
                            q0 = max(g, 0) * 128
                            n = QW - q0
                            spsm = ps2.tile([128, 512], F32, tag="s", bufs=3)
                            nc.tensor.matmul(
                                spsm[:, 0:n],
                                lhsT=krope_sb[:, 128 * j : 128 * (j + 1)],
                                rhs=qsrc[0:64, QW * i4 + q0 : QW * (i4 + 1)],
                                start=True,
                                stop=True,
                            )
                            pt = patt.tile([128, 512], F32R, tag="pt", bufs=3)
                            nc.scalar.activation(out=pt[:, 0:n], in_=spsm[:, 0:n], func=EXP, scale=0.125)
                            if g >= 0:
                                nc.vector.tensor_mul(pt[:, 0:128], pt[:, 0:128], mk_sb)
                            nc.tensor.matmul(
                                yps[:, q0:QW],
                                lhsT=vaug_sb[:, j, :],
                                rhs=pt[:, 0:n],
                                start=(j == 0),
                                stop=(j == ktiles - 1),
                                skip_group_check=True,
                            )
                        # epilogue: recip of the ones-row sums, broadcast, normalize
                        r_sb = patt.tile([65, QW], F32R, tag="r", bufs=2)
                        nc.vector.reciprocal(out=r_sb[64:65, :], in_=yps[64:65, :])
                        rbps = ps2.tile([64, QW], F32, tag="rb", bufs=1)
                        nc.tensor.matmul(rbps, lhsT=ones_sb[64:65, 0:64], rhs=r_sb[64:65, :], start=True, stop=True)
                        rb_sb = patt.tile([64, QW], F32, tag="rb_sb", bufs=2)
                        nc.scalar.copy(out=rb_sb, in_=rbps)
                        ynt = patt.tile([64, QW], F32R, tag=f"yn{h}", bufs=2)
                        nc.vector.tensor_mul(ynt, yps[0:64, :], rb_sb)
                        yn[h] = ynt
                    # proj for this window's 128-row tiles
                    for t4 in range(QW // 128):
                        tglob = i4 * (QW // 128) + t4
                        osb = patt.tile([128, C], F32, tag="o", bufs=3)
                        for n2 in range(2):
                            ops_ = ps2.tile([128, 512], F32, tag="o", bufs=2)
                            nc.tensor.matmul(
                                ops_,
                                lhsT=yn[0][:, 128 * t4 : 128 * (t4 + 1)],
                                rhs=wp0r[:, 512 * n2 : 512 * (n2 + 1)],
                                start=True,
                                stop=False,
                                skip_group_check=True,
                            )
                            nc.tensor.matmul(
                                ops_,
                                lhsT=yn[1][:, 128 * t4 : 128 * (t4 + 1)],
                                rhs=wp1r[:, 512 * n2 : 512 * (n2 + 1)],
                                start=False,
                                stop=True,
                                skip_group_check=True,
                            )
                            if n2 == 0:
                                nc.vector.tensor_copy(out=osb[:, 0:512], in_=ops_)
                            else:
                                nc.scalar.copy(out=osb[:, 512:1024], in_=ops_)
                        nc.sync.dma_start(out=out[128 * tglob : 128 * (tglob + 1), :], in_=osb)
    nc.finalize()
    return nc


def _get_nc() -> bass.Bass:
    if "nc" not in _CACHE:
        _CACHE["nc"] = _build_bass()
    return _CACHE["nc"]


def _make_in_maps(x, w_attn, w_proj):
    x2 = np.ascontiguousarray(np.asarray(x, dtype=np.float32).reshape(T, C).T)  # [C, T]
    wr = np.asarray(w_attn, dtype=np.float32).reshape(C, 4, 6, D)
    wp = np.asarray(w_proj, dtype=np.float32)
    in_maps = []
    for c in range(8):
        g = c // 2
        s = (2 * c) % 4
        wqkv_c = np.ascontiguousarray(
            np.concatenate([wr[:, g, s, :], wr[:, g, s + 1, :], wr[:, g, 4, :], wr[:, g, 5, :]], axis=1)
        )
        wproj_c = np.ascontiguousarray(wp[128 * c : 128 * (c + 1), :])
        in_maps.append({"xt": x2, "wqkv": wqkv_c, "wproj": wproj_c})
    return in_maps


def _combine(results):
    acc = np.zeros((T, C), np.float64)
    for r in results:
        acc += r["out"]
    return acc.astype(np.float32).reshape(1, T, C)


def run_for_test(inputs, trace=False):
    """Returns (output, exec_time_ns_or_None). Used by test.py."""
    nc = _get_nc()
    in_maps = _make_in_maps(**inputs)
    res = run_bass_kernel_spmd(nc, in_maps, core_ids=list(range(8)), trace=trace)
    return _combine(res.results), res.exec_time_ns


def kernel(x, w_attn, w_proj):
    out, _ = run_for_test({"x": x, "w_attn": w_attn, "w_proj": w_proj})
    return out


# revision 28
# speedup vs baseline: 1.1938x; 1.1938x over previous
"""Causal GQA self-attention (B=1, T=2048, C=1024, 16 q-heads, 4 kv-groups, d=64)
on 8 Trainium2 NeuronCores.

Sharding: tensor-parallel over heads. Core c owns q-heads (2c, 2c+1) and kv-group
c//2. Each core computes x @ w_attn for its slice (transposed layout), RoPE,
causal flash-style attention for its 2 heads, and its partial y @ w_proj
(contracting only its 128 head-dims). Host sums the 8 partial outputs.

Layout strategy (per core):
  - xT [C, T] in SBUF (strided DMA); wqkv slice [C, 256] natural.
  - qkvT = wqkv.T @ x computed transposed: qT2 [128, T] (2 heads), kvT [128, T]
    (k rows 0:64, v rows 64:128).
  - RoPE applied in [d, T] layout; the pair-rotation is a PE matmul with a
    constant +-1 permutation matrix; cos/sin are inline (baked) tables.
  - Scores computed TRANSPOSED: sT[k, q] = kT.T-free matmul, so the softmax
    denominator comes from appending a ones-column to v (one extra PE row) and
    no max-subtraction is needed (|scores| <= ~7, exp is safe in fp32).
  - att @ v computed as yT[d, q] via lhsT=v_aug, rhs=pT -- no transposes of p.
  - Normalization folded in before proj via a PE outer-product broadcast.
  - proj: out[t, :] += yT_h.T @ wproj_h per head, accumulated in PSUM.
"""

import numpy as np

import concourse.bass as bass
import concourse.mybir as mybir
from concourse import bacc
import concourse.tile as tile
from concourse.bass_utils import run_bass_kernel_spmd

T = 2048
C = 1024
D = 64
QW = 1024                     # queries processed per attention window
NCH = T // 512                # 512-wide column chunks of T
F32 = mybir.dt.float32
F32R = mybir.dt.float32r
EXP = mybir.ActivationFunctionType.Exp
MUL = mybir.AluOpType.mult
ADD = mybir.AluOpType.add

# True: load xT via strided DMA (512B contiguous runs in DRAM).
# False: load x naturally and transpose 128x128 blocks on the PE.
STRIDED_XT = True

_CACHE: dict = {}


def _rope_tables():
    # Replicate reference.apply_rope's f32 pipeline exactly.
    inv = (1.0 / (np.float32(10000.0) ** (np.arange(0, D, 2, dtype=np.float32) / np.float32(D)))).astype(np.float32)
    freqs = (np.arange(T, dtype=np.float32)[:, None] * inv[None, :]).astype(np.float32)  # (T, 32)
    freqs = np.repeat(freqs, 2, axis=1)                                                  # (T, 64)
    cos = np.cos(freqs).astype(np.float32).T.copy()                                      # (64, T)
    sin = np.sin(freqs).astype(np.float32).T.copy()
    cos2 = np.ascontiguousarray(np.concatenate([cos, cos], axis=0))                      # (128, T)
    sin2 = np.ascontiguousarray(np.concatenate([sin, sin], axis=0))
    return cos2, sin2


def _const_mats():
    # perm (as lhsT): rot[2i] = -x[2i+1], rot[2i+1] = +x[2i]
    perm = np.zeros((128, 128), np.float32)
    for i in range(64):
        perm[2 * i + 1, 2 * i] = -1.0
        perm[2 * i, 2 * i + 1] = 1.0
    ident = np.eye(128, dtype=np.float32)
    shift = np.zeros((128, 128), np.float32)   # [64+i, i] = 1 down-shift; [i, 64+i] = 1 up-shift
    for i in range(64):
        shift[64 + i, i] = 1.0
        shift[i, 64 + i] = 1.0
    kq = np.arange(128)
    binmask = np.where(kq[:, None] <= kq[None, :], 0.0, -8e29).astype(np.float32)  # [k, q] additive
    return perm, ident, shift, binmask


def _build_bass() -> bass.Bass:
    nc = bacc.Bacc(None, target_bir_lowering=False)
    xt_d = nc.dram_tensor("xt", [C, T], F32R, kind="ExternalInput")
    wqkv = nc.dram_tensor("wqkv", [C, 4 * D], F32R, kind="ExternalInput")
    wproj = nc.dram_tensor("wproj", [2 * D, C], F32R, kind="ExternalInput")
    out = nc.dram_tensor("out", [T, C], F32, kind="ExternalOutput")

    cos2_np, sin2_np = _rope_tables()
    perm_np, ident_np, shift_np, binmask_np = _const_mats()
    cos_d = nc.inline_tensor(cos2_np, name="cos2")
    sin_d = nc.inline_tensor(sin2_np, name="sin2")
    perm_d = nc.inline_tensor(perm_np, name="permm")
    ident_d = nc.inline_tensor(ident_np, name="identm")
    shift_d = nc.inline_tensor(shift_np, name="shiftm")
    mask_d = nc.inline_tensor(binmask_np, name="binmaskm")

    with tile.TileContext(nc) as tc:
        with (
            nc.allow_low_precision(reason="fp32r rounding of matmul operands"),
            tc.tile_pool(name="const", bufs=1) as const,
            tc.tile_pool(name="big", bufs=1) as big,
            tc.tile_pool(name="work", bufs=1) as work,
            tc.tile_pool(name="ps", bufs=1, space="PSUM") as psp,
        ):
            SB = 4  # shared 1-bank psum slots

            # x^T and wqkv on the SP HWDGE queue, first in line
            w_r = const.tile([128, 8, 4 * D], F32R)
            nc.sync.dma_start(out=w_r, in_=wqkv.rearrange("(c p) n -> p c n", p=128))
            xt = big.tile([128, 8, T], F32R)  # x^T as 8 c-tiles (host-transposed input)
            for half in range(2):
                for c in range(8):
                    nc.sync.dma_start(
                        out=xt[:, c, 1024 * half : 1024 * (half + 1)],
                        in_=xt_d[128 * c : 128 * (c + 1), 1024 * half : 1024 * (half + 1)],
                    )

            # small constants on the gpsimd (SWDGE) queue so they don't delay xt
            wpc = const.tile([128, C], F32R)
            nc.gpsimd.dma_start(out=wpc, in_=wproj[:, :])
            cos_sb = const.tile([128, T], F32)
            nc.gpsimd.dma_start(out=cos_sb, in_=cos_d[:, :])
            sin_sb = const.tile([128, T], F32)
            nc.gpsimd.dma_start(out=sin_sb, in_=sin_d[:, :])
            perm_f = const.tile([128, 128], F32)
            nc.gpsimd.dma_start(out=perm_f, in_=perm_d[:, :])
            id_sb = const.tile([128, 128], F32)
            nc.gpsimd.dma_start(out=id_sb, in_=ident_d[:, :])
            sh_f = const.tile([128, 128], F32)
            nc.gpsimd.dma_start(out=sh_f, in_=shift_d[:, :])
            mk_f = const.tile([128, 128], F32)
            nc.gpsimd.dma_start(out=mk_f, in_=mask_d[:, :])
            mk_sb = const.tile([128, 128], F32R)
            nc.vector.tensor_copy(out=mk_sb, in_=mk_f)
            idr_sb = const.tile([128, 128], F32R)
            nc.vector.tensor_copy(out=idr_sb, in_=id_sb)
            perm_sb = const.tile([128, 128], F32R)
            nc.vector.tensor_copy(out=perm_sb, in_=perm_f)
            sh_sb = const.tile([128, 128], F32R)
            nc.vector.tensor_copy(out=sh_sb, in_=sh_f)
            ones_f = const.tile([128, 64], F32)
            nc.vector.memset(ones_f, 1.0)
            ones_sb = const.tile([65, 64], F32R)
            nc.vector.tensor_copy(out=ones_sb, in_=ones_f[0:65, :])

            # persistent per-core activations
            qrope_sb = big.tile([128, T], F32R)   # roped q, h0 rows 0:64, h1 rows 64:128
            q1_sb = big.tile([64, T], F32R)       # roped q of h1 shifted to partitions 0:64
            krope_sb = big.tile([64, T], F32R)
            vaug_sb = big.tile([128, 16, D + 1], F32R)  # v tiles + ones column

            # ------- stage 1, one 512-wide chunk of T at a time -------
            def stage1_chunk(nch):
                sl = slice(512 * nch, 512 * (nch + 1))
                qraw = work.tile([128, 512], F32R, tag="qraw", bufs=2, name=f"qraw{nch}")
                kvraw = work.tile([128, 512], F32R, tag="kvraw", bufs=2, name=f"kvraw{nch}")
                for m, dst in ((0, qraw), (1, kvraw)):
                    ps = psp.tile([128, 512], F32, tag="s", bufs=SB, name=f"qkv{nch}_{m}")
                    for c in range(8):
                        nc.tensor.matmul(
                            ps,
                            lhsT=w_r[:, c, 128 * m : 128 * (m + 1)],
                            rhs=xt[:, c, sl],
                            start=(c == 0),
                            stop=(c == 7),
                        )
                    nc.vector.tensor_copy(out=dst, in_=ps)
                tmp = work.tile([128, 512], F32, tag="tmp", bufs=2, name=f"tmp{nch}")
                tmpk = work.tile([64, 512], F32, tag="tmpk", bufs=2, name=f"tmpk{nch}")
                # rope q (both heads at once)
                rps = psp.tile([128, 512], F32, tag="s", bufs=SB, name=f"rot{nch}")
                nc.tensor.matmul(rps, lhsT=perm_sb, rhs=qraw, start=True, stop=True)
                nc.vector.tensor_mul(tmp, rps, sin_sb[:, sl])
                nc.vector.tensor_mul(qrope_sb[:, sl], qraw, cos_sb[:, sl])
                nc.vector.tensor_add(qrope_sb[:, sl], qrope_sb[:, sl], tmp)
                # rope k (rows 0:64 of kvraw)
                rpsk = psp.tile([64, 512], F32, tag="s", bufs=SB, name=f"rotk{nch}")
                nc.tensor.matmul(rpsk, lhsT=perm_sb[0:64, 0:64], rhs=kvraw[0:64, :], start=True, stop=True)
                nc.vector.tensor_mul(tmpk, rpsk, sin_sb[0:64, sl])
                nc.vector.tensor_mul(krope_sb[:, sl], kvraw[0:64, :], cos_sb[0:64, sl])
                nc.vector.tensor_add(krope_sb[:, sl], krope_sb[:, sl], tmpk)
                # shift roped h1 q down to partitions 0:64
                sps = psp.tile([64, 512], F32, tag="s", bufs=SB, name=f"shift{nch}")
                nc.tensor.matmul(sps, lhsT=sh_sb[64:128, 0:64], rhs=qrope_sb[64:128, sl], start=True, stop=True)
                nc.vector.tensor_copy(out=q1_sb[:, sl], in_=sps)
                # v_aug: transpose v tiles of this chunk, append ones column
                for tt in range(4 * nch, 4 * nch + 4):
                    vps = psp.tile([128, 64], F32, tag="s", bufs=SB, name=f"vtr{tt}")
                    nc.tensor.transpose(
                        vps,
                        in_=kvraw[64:128, 128 * (tt - 4 * nch) : 128 * (tt - 4 * nch + 1)].bitcast(F32),
                        identity=id_sb[64:128, 64:128],
                    )
                    nc.vector.tensor_copy(out=vaug_sb[:, tt, 0:64], in_=vps)
                    nc.vector.tensor_copy(out=vaug_sb[:, tt, 64:65], in_=ones_f[:, 0:1])

            # ------- one attention window of QW queries (both heads interleaved) -------
            NQT = T // QW
            KPW = QW // 128
            def attn_window(i8):
                yn2 = work.tile([128, QW], F32R, tag="yn2", bufs=2, name=f"yn2_{i8}")
                ktiles = KPW * i8 + KPW
                yps = {
                    h: psp.tile([65, QW], F32, tag=f"yt{h}", bufs=1, name=f"yps_{i8}_{h}")
                    for h in range(2)
                }
                last_j = {0: KPW * i8 + 3, 1: ktiles - 1}

                def epilogue_half(a2):
                    hsl = slice(512 * a2, 512 * (a2 + 1))
                    for h in range(2):
                        r_sb = work.tile([65, 512], F32R, tag="r", bufs=2, name=f"r{i8}_{h}_{a2}")
                        nc.vector.reciprocal(out=r_sb[64:65, :], in_=yps[h][64:65, hsl])
                        rbps = psp.tile([64, 512], F32, tag="s", bufs=SB, name=f"rbp{i8}_{h}_{a2}")
                        nc.tensor.matmul(
                            rbps,
                            lhsT=ones_sb[64:65, 0:64],
                            rhs=r_sb[64:65, :],
                            start=True,
                            stop=True,
                        )
                        rb_sb = work.tile([64, 512], F32, tag="rb_sb", bufs=2, name=f"rb{i8}_{h}_{a2}")
                        nc.scalar.copy(out=rb_sb, in_=rbps)
                        if h == 0:
                            nc.vector.tensor_mul(yn2[0:64, hsl], yps[h][0:64, hsl], rb_sb)
                        else:
                            yn1 = work.tile([64, 512], F32R, tag="yn1", bufs=2, name=f"yn1_{i8}_{a2}")
                            nc.vector.tensor_mul(yn1, yps[h][0:64, hsl], rb_sb)
                            nc.gpsimd.dma_start(out=yn2[64:128, hsl], in_=yn1)
                    for t4 in range(4 * a2, 4 * a2 + 4):
                        tglob = i8 * (QW // 128) + t4
                        osb = work.tile([128, C], F32, tag="o", bufs=3, name=f"o{i8}_{t4}")
                        for n2 in range(2):
                            ops_ = psp.tile([128, 512], F32, tag="s", bufs=SB, name=f"op{i8}_{t4}_{n2}")
                            nc.tensor.matmul(
                                ops_,
                                lhsT=yn2[:, 128 * t4 : 128 * (t4 + 1)],
                                rhs=wpc[:, 512 * n2 : 512 * (n2 + 1)],
                                start=True,
                                stop=True,
                            )
                            if n2 == 0:
                                nc.vector.tensor_copy(out=osb[:, 0:512], in_=ops_)
                            else:
                                nc.scalar.copy(out=osb[:, 512:1024], in_=ops_)
                        nc.sync.dma_start(out=out[128 * tglob : 128 * (tglob + 1), :], in_=osb)

                prev = None
                for j in range(ktiles):
                    g = j - KPW * i8
                    q0 = max(g, 0) * 128
                    pts = {}
                    for h in range(2):
                        qsrc = qrope_sb if h == 0 else q1_sb
                        pt = work.tile([128, QW], F32R, tag="pt", bufs=5, name=f"pt{i8}_{j}_{h}")
                        for a2 in range(q0 // 512, 2):
                            lo = max(q0, 512 * a2)
                            hi = 512 * (a2 + 1)
                            spsm = psp.tile([128, 512], F32, tag="s", bufs=SB, name=f"s{i8}_{j}_{h}_{a2}")
                            nc.tensor.matmul(
                                spsm[:, 0 : hi - lo],
                                lhsT=krope_sb[:, 128 * j : 128 * (j + 1)],
                                rhs=qsrc[0:64, QW * i8 + lo : QW * i8 + hi],
                                start=True,
                                stop=not (lo <= q0 < hi and g >= 0),
                                skip_group_check=True,
                            )
                            if g >= 0 and lo <= q0 < hi:
                                nc.tensor.matmul(
                                    spsm[:, q0 - lo : q0 - lo + 128],
                                    lhsT=idr_sb,
                                    rhs=mk_sb,
                                    start=False,
                                    stop=True,
                                    skip_group_check=True,
                                )
                            nc.scalar.activation(
                                out=pt[:, lo:hi], in_=spsm[:, 0 : hi - lo], func=EXP, scale=0.125
                            )
                        pts[h] = pt

                    def emit_yt(jj, ptsj):
                        gg = jj - KPW * i8
                        qq0 = max(gg, 0) * 128
                        for h in range(2):
                            for a2 in range(qq0 // 512, 2):
                                lo = max(qq0, 512 * a2)
                                hi = 512 * (a2 + 1)
                                nc.tensor.matmul(
                                    yps[h][:, lo:hi],
                                    lhsT=vaug_sb[:, jj, :],
                                    rhs=ptsj[h][:, lo:hi],
                                    start=(jj == 0),
                                    stop=(jj == last_j[a2]),
                                    skip_group_check=True,
                                )

                    if prev is not None:
                        emit_yt(*prev)
                    prev = (j, pts)
                emit_yt(*prev)
                epilogue_half(0)
                epilogue_half(1)

            stage1_chunk(0)
            stage1_chunk(1)
            attn_window(0)
            stage1_chunk(2)
            stage1_chunk(3)
            attn_window(1)
    nc.finalize()
    return nc


def _get_nc() -> bass.Bass:
    if "nc" not in _CACHE:
        _CACHE["nc"] = _build_bass()
    return _CACHE["nc"]


def _make_in_maps(x, w_attn, w_proj):
    x2 = np.ascontiguousarray(np.asarray(x, dtype=np.float32).reshape(T, C).T)  # [C, T]
    wr = np.asarray(w_attn, dtype=np.float32).reshape(C, 4, 6, D)
    wp = np.asarray(w_proj, dtype=np.float32)
    in_maps = []
    for c in range(8):
        g = c // 2
        s = (2 * c) % 4
        wqkv_c = np.ascontiguousarray(
            np.concatenate([wr[:, g, s, :], wr[:, g, s + 1, :], wr[:, g, 4, :], wr[:, g, 5, :]], axis=1)
        )
        wproj_c = np.ascontiguousarray(wp[128 * c : 128 * (c + 1), :])
        in_maps.append({"xt": x2, "wqkv": wqkv_c, "wproj": wproj_c})
    return in_maps


def _combine(results):
    acc = np.zeros((T, C), np.float64)
    for r in results:
        acc += r["out"]
    return acc.astype(np.float32).reshape(1, T, C)


def run_for_test(inputs, trace=False):
    """Returns (output, exec_time_ns_or_None). Used by test.py."""
    nc = _get_nc()
    in_maps = _make_in_maps(**inputs)
    res = run_bass_kernel_spmd(nc, in_maps, core_ids=list(range(8)), trace=trace)
    return _combine(res.results), res.exec_time_ns


def kernel(x, w_attn, w_proj):
    out, _ = run_for_test({"x": x, "w_attn": w_attn, "w_proj": w_proj})
    return out


# revision 29
# speedup vs baseline: 79.1614x; 66.3084x over previous
"""Causal GQA self-attention (B=1, T=2048, C=1024, 16 q-heads, 4 kv-groups, d=64)
on 8 Trainium2 NeuronCores.

Sharding: tensor-parallel over heads. Core c owns q-heads (2c, 2c+1) and kv-group
c//2. Each core computes x @ w_attn for its slice (transposed layout), RoPE,
causal flash-style attention for its 2 heads, and its partial y @ w_proj
(contracting only its 128 head-dims). Host sums the 8 partial outputs.

Layout strategy (per core):
  - xT [C, T] in SBUF (strided DMA); wqkv slice [C, 256] natural.
  - qkvT = wqkv.T @ x computed transposed: qT2 [128, T] (2 heads), kvT [128, T]
    (k rows 0:64, v rows 64:128).
  - RoPE applied in [d, T] layout; the pair-rotation is a PE matmul with a
    constant +-1 permutation matrix; cos/sin are inline (baked) tables.
  - Scores computed TRANSPOSED: sT[k, q] = kT.T-free matmul, so the softmax
    denominator comes from appending a ones-column to v (one extra PE row) and
    no max-subtraction is needed (|scores| <= ~7, exp is safe in fp32).
  - att @ v computed as yT[d, q] via lhsT=v_aug, rhs=pT -- no transposes of p.
  - Normalization folded in before proj via a PE outer-product broadcast.
  - proj: out[t, :] += yT_h.T @ wproj_h per head, accumulated in PSUM.
"""

import numpy as np

import concourse.bass as bass
import concourse.mybir as mybir
from concourse import bacc
import concourse.tile as tile
from concourse.bass_utils import run_bass_kernel_spmd

T = 2048
C = 1024
D = 64
QW = 1024                     # queries processed per attention window
NCH = T // 512                # 512-wide column chunks of T
F32 = mybir.dt.float32
F32R = mybir.dt.float32r
EXP = mybir.ActivationFunctionType.Exp
MUL = mybir.AluOpType.mult
ADD = mybir.AluOpType.add

# True: load xT via strided DMA (512B contiguous runs in DRAM).
# False: load x naturally and transpose 128x128 blocks on the PE.
STRIDED_XT = True

_CACHE: dict = {}


def _rope_tables():
    # Replicate reference.apply_rope's f32 pipeline exactly.
    inv = (1.0 / (np.float32(10000.0) ** (np.arange(0, D, 2, dtype=np.float32) / np.float32(D)))).astype(np.float32)
    freqs = (np.arange(T, dtype=np.float32)[:, None] * inv[None, :]).astype(np.float32)  # (T, 32)
    freqs = np.repeat(freqs, 2, axis=1)                                                  # (T, 64)
    cos = np.cos(freqs).astype(np.float32).T.copy()                                      # (64, T)
    sin = np.sin(freqs).astype(np.float32).T.copy()
    cos2 = np.ascontiguousarray(np.concatenate([cos, cos], axis=0))                      # (128, T)
    sin2 = np.ascontiguousarray(np.concatenate([sin, sin], axis=0))
    return cos2, sin2


def _const_mats():
    # perm (as lhsT): rot[2i] = -x[2i+1], rot[2i+1] = +x[2i]
    perm = np.zeros((128, 128), np.float32)
    for i in range(64):
        perm[2 * i + 1, 2 * i] = -1.0
        perm[2 * i, 2 * i + 1] = 1.0
    ident = np.eye(128, dtype=np.float32)
    shift = np.zeros((128, 128), np.float32)   # [64+i, i] = 1 down-shift; [i, 64+i] = 1 up-shift
    for i in range(64):
        shift[64 + i, i] = 1.0
        shift[i, 64 + i] = 1.0
    kq = np.arange(128)
    binmask = np.where(kq[:, None] <= kq[None, :], 0.0, -8e29).astype(np.float32)  # [k, q] additive
    return perm, ident, shift, binmask


def _build_bass(repeat: int = 1) -> bass.Bass:
    nc = bacc.Bacc(None, target_bir_lowering=False)
    xt_d = nc.dram_tensor("xt", [C, T], F32R, kind="ExternalInput")
    wqkv = nc.dram_tensor("wqkv", [C, 4 * D], F32R, kind="ExternalInput")
    wproj = nc.dram_tensor("wproj", [2 * D, C], F32R, kind="ExternalInput")
    out = nc.dram_tensor("out", [T, C], F32, kind="ExternalOutput")

    cos2_np, sin2_np = _rope_tables()
    perm_np, ident_np, shift_np, binmask_np = _const_mats()
    cos_d = nc.inline_tensor(cos2_np, name="cos2")
    sin_d = nc.inline_tensor(sin2_np, name="sin2")
    perm_d = nc.inline_tensor(perm_np, name="permm")
    ident_d = nc.inline_tensor(ident_np, name="identm")
    shift_d = nc.inline_tensor(shift_np, name="shiftm")
    mask_d = nc.inline_tensor(binmask_np, name="binmaskm")

    with tile.TileContext(nc) as tc:
        with (
            nc.allow_low_precision(reason="fp32r rounding of matmul operands"),
            tc.tile_pool(name="const", bufs=1) as const,
            tc.tile_pool(name="big", bufs=1) as big,
            tc.tile_pool(name="work", bufs=1) as work,
            tc.tile_pool(name="ps", bufs=1, space="PSUM") as psp,
        ):
            SB = 4  # shared 1-bank psum slots

            # x^T and wqkv on the SP HWDGE queue, first in line
            w_r = const.tile([128, 8, 4 * D], F32R)
            nc.sync.dma_start(out=w_r, in_=wqkv.rearrange("(c p) n -> p c n", p=128))
            xt = big.tile([128, 8, T], F32R)  # x^T as 8 c-tiles (host-transposed input)

            def load_xt():
                for half in range(2):
                    for c in range(8):
                        nc.sync.dma_start(
                            out=xt[:, c, 1024 * half : 1024 * (half + 1)],
                            in_=xt_d[128 * c : 128 * (c + 1), 1024 * half : 1024 * (half + 1)],
                        )

            # small constants on the gpsimd (SWDGE) queue so they don't delay xt
            wpc = const.tile([128, C], F32R)
            nc.gpsimd.dma_start(out=wpc, in_=wproj[:, :])
            cos_sb = const.tile([128, T], F32)
            nc.gpsimd.dma_start(out=cos_sb, in_=cos_d[:, :])
            sin_sb = const.tile([128, T], F32)
            nc.gpsimd.dma_start(out=sin_sb, in_=sin_d[:, :])
            perm_f = const.tile([128, 128], F32)
            nc.gpsimd.dma_start(out=perm_f, in_=perm_d[:, :])
            id_sb = const.tile([128, 128], F32)
            nc.gpsimd.dma_start(out=id_sb, in_=ident_d[:, :])
            sh_f = const.tile([128, 128], F32)
            nc.gpsimd.dma_start(out=sh_f, in_=shift_d[:, :])
            mk_f = const.tile([128, 128], F32)
            nc.gpsimd.dma_start(out=mk_f, in_=mask_d[:, :])
            mk_sb = const.tile([128, 128], F32R)
            nc.vector.tensor_copy(out=mk_sb, in_=mk_f)
            idr_sb = const.tile([128, 128], F32R)
            nc.vector.tensor_copy(out=idr_sb, in_=id_sb)
            perm_sb = const.tile([128, 128], F32R)
            nc.vector.tensor_copy(out=perm_sb, in_=perm_f)
            sh_sb = const.tile([128, 128], F32R)
            nc.vector.tensor_copy(out=sh_sb, in_=sh_f)
            ones_f = const.tile([128, 64], F32)
            nc.vector.memset(ones_f, 1.0)
            ones_sb = const.tile([65, 64], F32R)
            nc.vector.tensor_copy(out=ones_sb, in_=ones_f[0:65, :])

            # persistent per-core activations
            qrope_sb = big.tile([128, T], F32R)   # roped q, h0 rows 0:64, h1 rows 64:128
            q1_sb = big.tile([64, T], F32R)       # roped q of h1 shifted to partitions 0:64
            krope_sb = big.tile([64, T], F32R)
            vaug_sb = big.tile([128, 16, D + 1], F32R)  # v tiles + ones column

            # ------- stage 1, one 512-wide chunk of T at a time -------
            rp = [0]

            def stage1_chunk(nch):
                sl = slice(512 * nch, 512 * (nch + 1))
                qraw = work.tile([128, 512], F32R, tag="qraw", bufs=2, name=f"x{rp[0]}qraw{nch}")
                kvraw = work.tile([128, 512], F32R, tag="kvraw", bufs=2, name=f"x{rp[0]}kvraw{nch}")
                for m, dst in ((0, qraw), (1, kvraw)):
                    ps = psp.tile([128, 512], F32, tag="s", bufs=SB, name=f"x{rp[0]}qkv{nch}_{m}")
                    for c in range(8):
                        nc.tensor.matmul(
                            ps,
                            lhsT=w_r[:, c, 128 * m : 128 * (m + 1)],
                            rhs=xt[:, c, sl],
                            start=(c == 0),
                            stop=(c == 7),
                        )
                    nc.vector.tensor_copy(out=dst, in_=ps)
                tmp = work.tile([128, 512], F32, tag="tmp", bufs=2, name=f"x{rp[0]}tmp{nch}")
                tmpk = work.tile([64, 512], F32, tag="tmpk", bufs=2, name=f"x{rp[0]}tmpk{nch}")
                # rope q (both heads at once)
                rps = psp.tile([128, 512], F32, tag="s", bufs=SB, name=f"x{rp[0]}rot{nch}")
                nc.tensor.matmul(rps, lhsT=perm_sb, rhs=qraw, start=True, stop=True)
                nc.vector.tensor_mul(tmp, rps, sin_sb[:, sl])
                nc.vector.tensor_mul(qrope_sb[:, sl], qraw, cos_sb[:, sl])
                nc.vector.tensor_add(qrope_sb[:, sl], qrope_sb[:, sl], tmp)
                # rope k (rows 0:64 of kvraw)
                rpsk = psp.tile([64, 512], F32, tag="s", bufs=SB, name=f"x{rp[0]}rotk{nch}")
                nc.tensor.matmul(rpsk, lhsT=perm_sb[0:64, 0:64], rhs=kvraw[0:64, :], start=True, stop=True)
                nc.vector.tensor_mul(tmpk, rpsk, sin_sb[0:64, sl])
                nc.vector.tensor_mul(krope_sb[:, sl], kvraw[0:64, :], cos_sb[0:64, sl])
                nc.vector.tensor_add(krope_sb[:, sl], krope_sb[:, sl], tmpk)
                # shift roped h1 q down to partitions 0:64
                sps = psp.tile([64, 512], F32, tag="s", bufs=SB, name=f"x{rp[0]}shift{nch}")
                nc.tensor.matmul(sps, lhsT=sh_sb[64:128, 0:64], rhs=qrope_sb[64:128, sl], start=True, stop=True)
                nc.vector.tensor_copy(out=q1_sb[:, sl], in_=sps)
                # v_aug: transpose v tiles of this chunk, append ones column
                for tt in range(4 * nch, 4 * nch + 4):
                    vps = psp.tile([128, 64], F32, tag="s", bufs=SB, name=f"x{rp[0]}vtr{tt}")
                    nc.tensor.transpose(
                        vps,
                        in_=kvraw[64:128, 128 * (tt - 4 * nch) : 128 * (tt - 4 * nch + 1)].bitcast(F32),
                        identity=id_sb[64:128, 64:128],
                    )
                    nc.vector.tensor_copy(out=vaug_sb[:, tt, 0:64], in_=vps)
                    nc.vector.tensor_copy(out=vaug_sb[:, tt, 64:65], in_=ones_f[:, 0:1])

            # ------- one attention window of QW queries (both heads interleaved) -------
            NQT = T // QW
            KPW = QW // 128
            def attn_window(i8):
                yn2 = work.tile([128, QW], F32R, tag="yn2", bufs=2, name=f"x{rp[0]}yn2_{i8}")
                ktiles = KPW * i8 + KPW
                yps = {
                    h: psp.tile([65, QW], F32, tag=f"yt{h}", bufs=1, name=f"x{rp[0]}yps_{i8}_{h}")
                    for h in range(2)
                }
                last_j = {0: KPW * i8 + 3, 1: ktiles - 1}

                def epilogue_half(a2):
                    hsl = slice(512 * a2, 512 * (a2 + 1))
                    for h in range(2):
                        r_sb = work.tile([65, 512], F32R, tag="r", bufs=2, name=f"x{rp[0]}r{i8}_{h}_{a2}")
                        nc.vector.reciprocal(out=r_sb[64:65, :], in_=yps[h][64:65, hsl])
                        rbps = psp.tile([64, 512], F32, tag="s", bufs=SB, name=f"x{rp[0]}rbp{i8}_{h}_{a2}")
                        nc.tensor.matmul(
                            rbps,
                            lhsT=ones_sb[64:65, 0:64],
                            rhs=r_sb[64:65, :],
                            start=True,
                            stop=True,
                        )
                        rb_sb = work.tile([64, 512], F32, tag="rb_sb", bufs=2, name=f"x{rp[0]}rb{i8}_{h}_{a2}")
                        nc.scalar.copy(out=rb_sb, in_=rbps)
                        if h == 0:
                            nc.vector.tensor_mul(yn2[0:64, hsl], yps[h][0:64, hsl], rb_sb)
                        else:
                            yn1 = work.tile([64, 512], F32R, tag="yn1", bufs=2, name=f"x{rp[0]}yn1_{i8}_{a2}")
                            nc.vector.tensor_mul(yn1, yps[h][0:64, hsl], rb_sb)
                            nc.gpsimd.dma_start(out=yn2[64:128, hsl], in_=yn1)
                    for t4 in range(4 * a2, 4 * a2 + 4):
                        tglob = i8 * (QW // 128) + t4
                        osb = work.tile([128, C], F32, tag="o", bufs=3, name=f"x{rp[0]}o{i8}_{t4}")
                        for n2 in range(2):
                            ops_ = psp.tile([128, 512], F32, tag="s", bufs=SB, name=f"x{rp[0]}op{i8}_{t4}_{n2}")
                            nc.tensor.matmul(
                                ops_,
                                lhsT=yn2[:, 128 * t4 : 128 * (t4 + 1)],
                                rhs=wpc[:, 512 * n2 : 512 * (n2 + 1)],
                                start=True,
                                stop=True,
                            )
                            if n2 == 0:
                                nc.vector.tensor_copy(out=osb[:, 0:512], in_=ops_)
                            else:
                                nc.scalar.copy(out=osb[:, 512:1024], in_=ops_)
                        nc.sync.dma_start(out=out[128 * tglob : 128 * (tglob + 1), :], in_=osb)

                prev = None
                for j in range(ktiles):
                    g = j - KPW * i8
                    q0 = max(g, 0) * 128
                    pts = {}
                    for h in range(2):
                        qsrc = qrope_sb if h == 0 else q1_sb
                        pt = work.tile([128, QW], F32R, tag="pt", bufs=5, name=f"x{rp[0]}pt{i8}_{j}_{h}")
                        for a2 in range(q0 // 512, 2):
                            lo = max(q0, 512 * a2)
                            hi = 512 * (a2 + 1)
                            spsm = psp.tile([128, 512], F32, tag="s", bufs=SB, name=f"x{rp[0]}s{i8}_{j}_{h}_{a2}")
                            nc.tensor.matmul(
                                spsm[:, 0 : hi - lo],
                                lhsT=krope_sb[:, 128 * j : 128 * (j + 1)],
                                rhs=qsrc[0:64, QW * i8 + lo : QW * i8 + hi],
                                start=True,
                                stop=not (lo <= q0 < hi and g >= 0),
                                skip_group_check=True,
                            )
                            if g >= 0 and lo <= q0 < hi:
                                nc.tensor.matmul(
                                    spsm[:, q0 - lo : q0 - lo + 128],
                                    lhsT=idr_sb,
                                    rhs=mk_sb,
                                    start=False,
                                    stop=True,
                                    skip_group_check=True,
                                )
                            nc.scalar.activation(
                                out=pt[:, lo:hi], in_=spsm[:, 0 : hi - lo], func=EXP, scale=0.125
                            )
                        pts[h] = pt

                    def emit_yt(jj, ptsj):
                        gg = jj - KPW * i8
                        qq0 = max(gg, 0) * 128
                        for h in range(2):
                            for a2 in range(qq0 // 512, 2):
                                lo = max(qq0, 512 * a2)
                                hi = 512 * (a2 + 1)
                                nc.tensor.matmul(
                                    yps[h][:, lo:hi],
                                    lhsT=vaug_sb[:, jj, :],
                                    rhs=ptsj[h][:, lo:hi],
                                    start=(jj == 0),
                                    stop=(jj == last_j[a2]),
                                    skip_group_check=True,
                                )

                    if prev is not None:
                        emit_yt(*prev)
                    prev = (j, pts)
                emit_yt(*prev)
                epilogue_half(0)
                epilogue_half(1)

            for _rep in range(repeat):
                rp[0] = _rep
                load_xt()
                stage1_chunk(0)
                stage1_chunk(1)
                attn_window(0)
                stage1_chunk(2)
                stage1_chunk(3)
                attn_window(1)
    nc.finalize()
    return nc


def _get_nc(repeat: int = 1) -> bass.Bass:
    key = ("nc", repeat)
    if key not in _CACHE:
        _CACHE[key] = _build_bass(repeat)
    return _CACHE[key]


def _make_in_maps(x, w_attn, w_proj):
    x2 = np.ascontiguousarray(np.asarray(x, dtype=np.float32).reshape(T, C).T)  # [C, T]
    wr = np.asarray(w_attn, dtype=np.float32).reshape(C, 4, 6, D)
    wp = np.asarray(w_proj, dtype=np.float32)
    in_maps = []
    for c in range(8):
        g = c // 2
        s = (2 * c) % 4
        wqkv_c = np.ascontiguousarray(
            np.concatenate([wr[:, g, s, :], wr[:, g, s + 1, :], wr[:, g, 4, :], wr[:, g, 5, :]], axis=1)
        )
        wproj_c = np.ascontiguousarray(wp[128 * c : 128 * (c + 1), :])
        in_maps.append({"xt": x2, "wqkv": wqkv_c, "wproj": wproj_c})
    return in_maps


def _combine(results):
    acc = np.zeros((T, C), np.float64)
    for r in results:
        acc += r["out"]
    return acc.astype(np.float32).reshape(1, T, C)


def run_for_test(inputs, trace=False):
    """Returns (output, exec_time_ns_or_None). Used by test.py."""
    nc = _get_nc()
    in_maps = _make_in_maps(**inputs)
    res = run_bass_kernel_spmd(nc, in_maps, core_ids=list(range(8)), trace=trace)
    return _combine(res.results), res.exec_time_ns


def kernel(x, w_attn, w_proj):
    out, _ = run_for_test({"x": x, "w_attn": w_attn, "w_proj": w_proj})
    return out
